# revision 34
# baseline (speedup 1.0000x reference)
"""Trainium2 Bass kernel for batched scaled-dot-product attention + 1x1-conv FFN.

Reference computation (per batch n of 4):
    S    = q @ k.T / 8           [P, P]   (P=4096, d_k=64)
    A    = softmax(S, axis=-1)
    out  = (A @ v) @ W.T + b     [P, 256]

Sharding: 8 cores = 4 batches x 2 query-halves (2048 queries each, full K/V).
No collectives needed; host scatters inputs / gathers outputs.

Per-core dataflow (flash-attention style, query tiles of 512, all matmuls
bf16 with fp32 PSUM accumulation):
    - S^T chunks [128kv, 512q] via TensorE matmuls; contraction d=64 is
      zero-padded to K=128 (host ships qT/kT with zero rows 64-127) — matmul
      time is N-cycles regardless of K, and S^T is PSUM-output-rate bound
    - exp on ScalarE, PSUM -> SBUF bf16, scale=1/8 fused into the activation;
      no max subtraction needed (scores/8 ~ N(0,1), exp cannot overflow)
    - A @ [V | 1]: exp^T chunks as the stationary operand over V augmented
      with a ones column, so the softmax denominator falls out of the same
      PSUM accumulation; deferred normalization (divide commutes with the FC)
    - per-partition reciprocal + scale on VectorE, attn^T via xbar
      DMA-transpose, FC against host-pretransposed W^T, bias added on VectorE
The software pipeline keeps TensorE >95% busy: S/exp run one iteration ahead
of the A@V bursts, and the previous tile's transpose/FC/store epilogue is
spread through the current tile's steady loop.
"""

import sys

sys.path.insert(0, "/opt/trn_rl_repo")

from contextlib import ExitStack

import ml_dtypes
import numpy as np

import concourse.tile as tile
from concourse import bacc, mybir

N_BATCH = 4
P_KV = 4096  # keys/values per batch
D_K = 64
D_V = 256
N_CORES = 8
Q_SHARD = N_BATCH * P_KV // N_CORES  # 2048 queries per core
QT = 512  # query tile width
N_QT = Q_SHARD // QT  # 4
N_SUB = QT // 128  # 4 query sub-tiles per tile
N_KC = P_KV // 128  # 32 kv chunks

F32 = mybir.dt.float32
BF16 = mybir.dt.bfloat16


def build_nc():
    nc = bacc.Bacc("TRN2", target_bir_lowering=False, debug=False)
    # q/k/w arrive host-transposed and bf16-cast: qt/kt are [128, N] with the
    # 64 d_k rows on top and zeros below (K=128 zero-padded contraction);
    # wt is W.T. Layout prep is part of the host-side sharding.
    q_d = nc.declare_dram_parameter("qt", [128, Q_SHARD], BF16, isOutput=False)
    k_d = nc.declare_dram_parameter("kt", [128, P_KV], BF16, isOutput=False)
    v_d = nc.declare_dram_parameter("v", [P_KV, D_V], BF16, isOutput=False)
    w_d = nc.declare_dram_parameter("wt", [D_V, D_V], BF16, isOutput=False)
    b_d = nc.declare_dram_parameter("b", [D_V], F32, isOutput=False)
    o_d = nc.declare_dram_parameter("out", [Q_SHARD, D_V], F32, isOutput=True)

    with tile.TileContext(nc) as tc, ExitStack() as ctx:
        persist = ctx.enter_context(tc.tile_pool(name="persist", bufs=1))
        stage = ctx.enter_context(tc.tile_pool(name="stage", bufs=1))
        sb_small = ctx.enter_context(tc.tile_pool(name="small", bufs=4))
        sb_attn = ctx.enter_context(tc.tile_pool(name="attn", bufs=6))
        sb_out = ctx.enter_context(tc.tile_pool(name="osb", bufs=6))
        sb_exp = ctx.enter_context(tc.tile_pool(name="exp", bufs=8))
        # PSUM: ps_s = 2 x [128,1024] (2 banks each) shared by S^T chunks and
        # the FC outputs; ps_o = 4 x [128,264] (1 bank each) for the 4
        # per-subtile attention accumulators. Total 8 banks.
        ps_s = ctx.enter_context(tc.tile_pool(name="ps_s", bufs=2, space="PSUM"))
        ps_o = ctx.enter_context(tc.tile_pool(name="ps_o", bufs=4, space="PSUM"))

        # ---- constants ----
        b_nat = persist.tile([1, D_V], F32, tag="b_nat")
        nc.sync.dma_start(out=b_nat, in_=b_d[:].unsqueeze(0))
        b_bcast = persist.tile([128, D_V], F32, tag="b_bcast")
        nc.gpsimd.partition_broadcast(b_bcast, b_nat)

        # ---- staging ----
        # split q/k loads so the first S-matmuls start after ~384KB, not 1.5MB
        qTs = []
        for tq in range(N_QT):
            qT_t = persist.tile([128, QT], BF16, tag=f"qT{tq}", name=f"qT{tq}")
            qTs.append(qT_t)
        kT_tiles = []
        kT_tiles.append(persist.tile([128, 512], BF16, tag="kTa", name="kTa"))
        kT_tiles.append(persist.tile([128, 512], BF16, tag="kTb", name="kTb"))
        for g in range(1, 4):
            kT_tiles.append(
                persist.tile([128, P_KV // 4], BF16, tag=f"kT{g}", name=f"kT{g}")
            )

        def kT_slice(c):
            if c < 4:
                return kT_tiles[0][:, c * 128 : (c + 1) * 128]
            if c < 8:
                return kT_tiles[1][:, (c - 4) * 128 : (c - 3) * 128]
            g = c // 8
            return kT_tiles[g + 1][:, (c % 8) * 128 : (c % 8 + 1) * 128]
        # chop loads into many DMA instructions — each lands on its own
        # queue (~22GB/s per queue), so splitting engages the full fabric
        def chop(eng, dst, srcv, lo, hi, n):
            step = (hi - lo) // n
            for i in range(n):
                a = lo + i * step
                eng.dma_start(out=dst[:, a - lo : a - lo + step], in_=srcv[:, a : a + step])

        chop(nc.scalar, qTs[0], q_d, 0, QT, 4)
        chop(nc.scalar, kT_tiles[0], k_d, 0, 512, 4)
        chop(nc.scalar, kT_tiles[1], k_d, 512, 1024, 4)
        for g in range(1, 4):
            chop(nc.sync, kT_tiles[g + 1], k_d, g * (P_KV // 4), (g + 1) * (P_KV // 4), 4)
        for tq in range(1, N_QT):
            chop(nc.sync, qTs[tq], q_d, tq * QT, (tq + 1) * QT, 2)
        wT = persist.tile([128, 2, D_V], BF16, tag="wT")
        nc.gpsimd.dma_start(
            out=wT, in_=w_d[:].rearrange("(cc p) o -> p cc o", p=128)
        )
        # V with ones columns: v_aug[p, chunk, 0:256]=v, [..., 256:264]=1
        # (264 = matmul moving-operand 16B alignment; column 256 is used)
        v_aug = persist.tile([128, N_KC, D_V + 8], BF16, tag="v_aug")
        v_re = v_d[:].rearrange("(c p) v -> p c v", p=128)
        for c0 in range(0, N_KC, 2):
            nc.gpsimd.dma_start(
                out=v_aug[:, c0 : c0 + 2, 0:D_V], in_=v_re[:, c0 : c0 + 2, :]
            )
        nc.vector.memset(v_aug[:, :, D_V : D_V + 8], 1.0)

        # warm the PE clock (HAM un-throttles after ~3.4us of activity)
        # during the initial DMA wait, so real matmuls start at 2.4 GHz
        warm = stage.tile([128, 512], BF16, tag="warm")
        nc.vector.memset(warm, 0.0)
        for _ in range(9):
            pw = ps_s.tile([128, 512], F32, tag="s", name="pw")
            nc.tensor.matmul(
                pw, lhsT=warm[:, 0:128], rhs=warm, start=True, stop=True
            )

        # ---- main loop over query tiles ----
        # The transpose+FC+out epilogue of tile t-1 is spread inside tile t's
        # steady loop so PE fills exp-wait gaps instead of a serial tail.
        tailstate = {}

        def emit_tail_piece(attn, qt_prev, s, piece):
            key = (qt_prev, s)
            if piece == 0:
                attnT = sb_attn.tile([128, 2, 128], BF16, tag="att", name="attnT")
                tailstate[key] = attnT
                nc.sync.dma_start(out=attnT, in_=attn, transpose=True)
            else:
                attnT = tailstate.pop(key)
                pf = ps_s.tile([128, D_V], F32, tag="s", name="pf")
                for cc in range(2):
                    nc.tensor.matmul(
                        pf,
                        lhsT=(attnT[:, cc, :]),
                        rhs=(wT[:, cc, :]),
                        start=(cc == 0),
                        stop=(cc == 1),
                    )
                osb = sb_out.tile([128, D_V], F32, tag="ou", name="osb")
                nc.vector.tensor_add(osb, pf, b_bcast)
                row0 = qt_prev * QT + s * 128
                nc.scalar.dma_start(out=o_d[row0 : row0 + 128, :], in_=osb)

        prev = None
        for qt in range(N_QT):
            po = [
                ps_o.tile([128, D_V + 8], F32, tag="o", name=f"po{s}")
                for s in range(N_SUB)
            ]
            expTs = {}

            def emit_s_exp(idx2):
                jj = 2 * idx2
                ps = ps_s.tile([128, 2 * QT], F32, tag="s", name="ps")
                expT = sb_exp.tile([128, 2 * QT], BF16, tag="expT", name="expT")
                expTs[jj] = expT
                for dj in range(2):
                    c = jj + dj
                    nc.tensor.matmul(
                        ps[:, dj * QT : (dj + 1) * QT],
                        lhsT=kT_slice(c),
                        rhs=qTs[qt],
                        start=True,
                        stop=True,
                    )
                nc.scalar.activation(
                    out=expT[:, :],
                    in_=ps[:, :],
                    func=mybir.ActivationFunctionType.Exp,
                    scale=0.125,
                )

            emit_s_exp(0)
            for idx in range(N_KC // 2):
                if idx + 1 < N_KC // 2:
                    emit_s_exp(idx + 1)
                jj = 2 * idx
                for dj in range(2):
                    j = jj + dj
                    for s in range(N_SUB):
                        nc.tensor.matmul(
                            po[s],
                            lhsT=expTs[jj][
                                :, dj * QT + s * 128 : dj * QT + (s + 1) * 128
                            ],
                            rhs=(v_aug[:, j, :]),
                            start=(j == 0),
                            stop=(j == N_KC - 1),
                        )
                if prev is not None and 2 <= idx < 2 + 2 * N_SUB:
                    p_attns, p_qt = prev
                    s, piece = divmod(idx - 2, 2)
                    emit_tail_piece(p_attns[s], p_qt, s, piece)

            attns = []
            for s in range(N_SUB):
                recip = sb_small.tile([128, 1], F32, tag="rc", name="recip")
                nc.vector.reciprocal(recip, po[s][:, D_V : D_V + 1])
                attn = sb_attn.tile([128, D_V], BF16, tag="at", name="attn")
                nc.vector.tensor_scalar_mul(attn, po[s][:, 0:D_V], recip)
                attns.append(attn)
            prev = (attns, qt)

        p_attns, p_qt = prev
        for s in range(N_SUB):
            for piece in range(2):
                emit_tail_piece(p_attns[s], p_qt, s, piece)

    nc.compile()
    return nc


_NC_CACHE = None


def _get_nc():
    global _NC_CACHE
    if _NC_CACHE is None:
        _NC_CACHE = build_nc()
    return _NC_CACHE


def _pad_t(x):
    xt = np.asarray(x).T.astype(ml_dtypes.bfloat16)
    out = np.zeros((128, xt.shape[1]), dtype=ml_dtypes.bfloat16)
    out[: xt.shape[0]] = xt
    return out


def make_in_maps(k_src, v_src, q_tgr, W_fc, b_fc):
    in_maps = []
    for core in range(N_CORES):
        n, h = divmod(core, 2)
        in_maps.append(
            {
                "qt": _pad_t(q_tgr[n, h * Q_SHARD : (h + 1) * Q_SHARD, :]),
                "kt": _pad_t(k_src[n]),
                "v": np.ascontiguousarray(np.asarray(v_src[n]).astype(ml_dtypes.bfloat16)),
                "wt": np.ascontiguousarray(
                    np.asarray(W_fc).T.astype(ml_dtypes.bfloat16)
                ),
                "b": np.ascontiguousarray(b_fc, dtype=np.float32),
            }
        )
    return in_maps


def assemble_out(results):
    out = np.empty((N_BATCH, P_KV, D_V), dtype=np.float32)
    for core in range(N_CORES):
        n, h = divmod(core, 2)
        out[n, h * Q_SHARD : (h + 1) * Q_SHARD, :] = results[core]["out"]
    return out


def kernel(k_src, v_src, q_tgr, W_fc, b_fc):
    from concourse.bass_utils import run_bass_kernel_spmd

    nc = _get_nc()
    in_maps = make_in_maps(k_src, v_src, q_tgr, W_fc, b_fc)
    res = run_bass_kernel_spmd(nc, in_maps, core_ids=list(range(N_CORES)))
    return assemble_out(res.results)


# revision 35
# speedup vs baseline: 1.0273x; 1.0273x over previous
"""Trainium2 Bass kernel for batched scaled-dot-product attention + 1x1-conv FFN.

Reference computation (per batch n of 4):
    S    = q @ k.T / 8           [P, P]   (P=4096, d_k=64)
    A    = softmax(S, axis=-1)
    out  = (A @ v) @ W.T + b     [P, 256]

Sharding: 8 cores = 4 batches x 2 query-halves (2048 queries each, full K/V).
No collectives needed; host scatters inputs / gathers outputs.

Per-core dataflow (flash-attention style, query tiles of 512, all matmuls
bf16 with fp32 PSUM accumulation):
    - S^T chunks [128kv, 512q] via TensorE matmuls; contraction d=64 is
      zero-padded to K=128 (host ships qT/kT with zero rows 64-127) — matmul
      time is N-cycles regardless of K, and S^T is PSUM-output-rate bound
    - exp on ScalarE, PSUM -> SBUF bf16, scale=1/8 fused into the activation;
      no max subtraction needed (scores/8 ~ N(0,1), exp cannot overflow)
    - A @ [V | 1]: exp^T chunks as the stationary operand over V augmented
      with a ones column, so the softmax denominator falls out of the same
      PSUM accumulation; deferred normalization (divide commutes with the FC)
    - per-partition reciprocal + scale on VectorE, attn^T via xbar
      DMA-transpose, FC against host-pretransposed W^T, bias added on VectorE
The software pipeline keeps TensorE >95% busy: S/exp run one iteration ahead
of the A@V bursts, and the previous tile's transpose/FC/store epilogue is
spread through the current tile's steady loop.
"""

import sys

sys.path.insert(0, "/opt/trn_rl_repo")

from contextlib import ExitStack

import ml_dtypes
import numpy as np

import concourse.tile as tile
from concourse import bacc, mybir

N_BATCH = 4
P_KV = 4096  # keys/values per batch
D_K = 64
D_V = 256
N_CORES = 8
Q_SHARD = N_BATCH * P_KV // N_CORES  # 2048 queries per core
QT = 512  # query tile width
N_QT = Q_SHARD // QT  # 4
N_SUB = QT // 128  # 4 query sub-tiles per tile
N_KC = P_KV // 128  # 32 kv chunks

F32 = mybir.dt.float32
BF16 = mybir.dt.bfloat16


def build_nc():
    nc = bacc.Bacc("TRN2", target_bir_lowering=False, debug=False)
    # q/k/w arrive host-transposed and bf16-cast: qt/kt are [128, N] with the
    # 64 d_k rows on top and zeros below (K=128 zero-padded contraction);
    # wt is W.T. Layout prep is part of the host-side sharding.
    q_d = nc.declare_dram_parameter("qt", [128, Q_SHARD], BF16, isOutput=False)
    k_d = nc.declare_dram_parameter("kt", [128, P_KV], BF16, isOutput=False)
    v_d = nc.declare_dram_parameter("v", [P_KV, D_V], BF16, isOutput=False)
    w_d = nc.declare_dram_parameter("wt", [D_V, D_V], BF16, isOutput=False)
    b_d = nc.declare_dram_parameter("b", [D_V], F32, isOutput=False)
    o_d = nc.declare_dram_parameter("out", [Q_SHARD, D_V], F32, isOutput=True)

    with tile.TileContext(nc) as tc, ExitStack() as ctx:
        persist = ctx.enter_context(tc.tile_pool(name="persist", bufs=1))
        stage = ctx.enter_context(tc.tile_pool(name="stage", bufs=1))
        sb_small = ctx.enter_context(tc.tile_pool(name="small", bufs=4))
        sb_attn = ctx.enter_context(tc.tile_pool(name="attn", bufs=6))
        sb_out = ctx.enter_context(tc.tile_pool(name="osb", bufs=6))
        sb_exp = ctx.enter_context(tc.tile_pool(name="exp", bufs=8))
        # PSUM: ps_s = 2 x [128,1024] (2 banks each) shared by S^T chunks and
        # the FC outputs; ps_o = 4 x [128,264] (1 bank each) for the 4
        # per-subtile attention accumulators. Total 8 banks.
        ps_s = ctx.enter_context(tc.tile_pool(name="ps_s", bufs=2, space="PSUM"))
        ps_o = ctx.enter_context(tc.tile_pool(name="ps_o", bufs=4, space="PSUM"))

        # ---- constants ----
        b_nat = persist.tile([1, D_V], F32, tag="b_nat")
        nc.sync.dma_start(out=b_nat, in_=b_d[:].unsqueeze(0))
        b_bcast = persist.tile([128, D_V], F32, tag="b_bcast")
        nc.gpsimd.partition_broadcast(b_bcast, b_nat)

        # ---- staging ----
        # split q/k loads so the first S-matmuls start after ~384KB, not 1.5MB
        qTs = []
        for tq in range(N_QT):
            qT_t = persist.tile([128, QT], BF16, tag=f"qT{tq}", name=f"qT{tq}")
            qTs.append(qT_t)
        kT_tiles = []
        kT_tiles.append(persist.tile([128, 512], BF16, tag="kTa", name="kTa"))
        kT_tiles.append(persist.tile([128, 512], BF16, tag="kTb", name="kTb"))
        for g in range(1, 4):
            kT_tiles.append(
                persist.tile([128, P_KV // 4], BF16, tag=f"kT{g}", name=f"kT{g}")
            )

        def kT_slice(c):
            if c < 4:
                return kT_tiles[0][:, c * 128 : (c + 1) * 128]
            if c < 8:
                return kT_tiles[1][:, (c - 4) * 128 : (c - 3) * 128]
            g = c // 8
            return kT_tiles[g + 1][:, (c % 8) * 128 : (c % 8 + 1) * 128]
        # chop loads into many DMA instructions — each lands on its own
        # queue (~22GB/s per queue), so splitting engages the full fabric
        def chop(eng, dst, srcv, lo, hi, n):
            step = (hi - lo) // n
            for i in range(n):
                a = lo + i * step
                eng.dma_start(out=dst[:, a - lo : a - lo + step], in_=srcv[:, a : a + step])

        chop(nc.scalar, qTs[0], q_d, 0, QT, 4)
        chop(nc.scalar, kT_tiles[0], k_d, 0, 512, 4)
        chop(nc.scalar, kT_tiles[1], k_d, 512, 1024, 4)
        for g in range(1, 4):
            chop(nc.sync, kT_tiles[g + 1], k_d, g * (P_KV // 4), (g + 1) * (P_KV // 4), 4)
        for tq in range(1, N_QT):
            chop(nc.sync, qTs[tq], q_d, tq * QT, (tq + 1) * QT, 2)
        wT = persist.tile([128, 2, D_V], BF16, tag="wT")
        nc.gpsimd.dma_start(
            out=wT, in_=w_d[:].rearrange("(cc p) o -> p cc o", p=128)
        )
        # V with ones columns: v_aug[p, chunk, 0:256]=v, [..., 256:264]=1
        # (264 = matmul moving-operand 16B alignment; column 256 is used)
        v_aug = persist.tile([128, N_KC, D_V + 8], BF16, tag="v_aug")
        v_re = v_d[:].rearrange("(c p) v -> p c v", p=128)
        for c0 in range(0, N_KC, 2):
            nc.gpsimd.dma_start(
                out=v_aug[:, c0 : c0 + 2, 0:D_V], in_=v_re[:, c0 : c0 + 2, :]
            )
        nc.vector.memset(v_aug[:, :, D_V : D_V + 8], 1.0)

        # warm the PE clock (HAM un-throttles after ~3.4us of activity)
        # during the initial DMA wait, so real matmuls start at 2.4 GHz
        warm = stage.tile([128, 512], BF16, tag="warm")
        nc.vector.memset(warm, 0.0)
        for _ in range(9):
            pw = ps_s.tile([128, 512], F32, tag="s", name="pw")
            nc.tensor.matmul(
                pw, lhsT=warm[:, 0:128], rhs=warm, start=True, stop=True
            )

        # ---- main loop over query tiles ----
        # The transpose+FC+out epilogue of tile t-1 is spread inside tile t's
        # steady loop so PE fills exp-wait gaps instead of a serial tail.
        tailstate = {}

        def emit_tail_piece(attn, qt_prev, s, piece):
            key = (qt_prev, s)
            if piece == 0:
                attnT = sb_attn.tile([128, 2, 128], BF16, tag="att", name="attnT")
                tailstate[key] = attnT
                nc.sync.dma_start(out=attnT, in_=attn, transpose=True)
            else:
                attnT = tailstate.pop(key)
                pf = ps_s.tile([128, D_V], F32, tag="s", name="pf")
                for cc in range(2):
                    nc.tensor.matmul(
                        pf,
                        lhsT=(attnT[:, cc, :]),
                        rhs=(wT[:, cc, :]),
                        start=(cc == 0),
                        stop=(cc == 1),
                    )
                osb = sb_out.tile([128, D_V], F32, tag="ou", name="osb")
                nc.vector.tensor_add(osb, pf, b_bcast)
                row0 = qt_prev * QT + s * 128
                nc.gpsimd.dma_start(out=o_d[row0 : row0 + 128, :], in_=osb)

        prev = None
        for qt in range(N_QT):
            po = [
                ps_o.tile([128, D_V + 8], F32, tag="o", name=f"po{s}")
                for s in range(N_SUB)
            ]
            expTs = {}

            def emit_s_exp(idx2):
                jj = 2 * idx2
                ps = ps_s.tile([128, 2 * QT], F32, tag="s", name="ps")
                expT = sb_exp.tile([128, 2 * QT], BF16, tag="expT", name="expT")
                expTs[jj] = expT
                for dj in range(2):
                    c = jj + dj
                    nc.tensor.matmul(
                        ps[:, dj * QT : (dj + 1) * QT],
                        lhsT=kT_slice(c),
                        rhs=qTs[qt],
                        start=True,
                        stop=True,
                    )
                nc.scalar.activation(
                    out=expT[:, :],
                    in_=ps[:, :],
                    func=mybir.ActivationFunctionType.Exp,
                    scale=0.125,
                )

            emit_s_exp(0)
            for idx in range(N_KC // 2):
                if idx + 1 < N_KC // 2:
                    emit_s_exp(idx + 1)
                jj = 2 * idx
                for dj in range(2):
                    j = jj + dj
                    for s in range(N_SUB):
                        nc.tensor.matmul(
                            po[s],
                            lhsT=expTs[jj][
                                :, dj * QT + s * 128 : dj * QT + (s + 1) * 128
                            ],
                            rhs=(v_aug[:, j, :]),
                            start=(j == 0),
                            stop=(j == N_KC - 1),
                        )
                if prev is not None and 2 <= idx < 2 + 2 * N_SUB:
                    p_attns, p_qt = prev
                    s, piece = divmod(idx - 2, 2)
                    emit_tail_piece(p_attns[s], p_qt, s, piece)

            attns = []
            for s in range(N_SUB):
                recip = sb_small.tile([128, 1], F32, tag="rc", name="recip")
                nc.vector.reciprocal(recip, po[s][:, D_V : D_V + 1])
                attn = sb_attn.tile([128, D_V], BF16, tag="at", name="attn")
                nc.vector.tensor_scalar_mul(attn, po[s][:, 0:D_V], recip)
                attns.append(attn)
            prev = (attns, qt)

        p_attns, p_qt = prev
        for s in range(N_SUB):
            for piece in range(2):
                emit_tail_piece(p_attns[s], p_qt, s, piece)

    nc.compile()
    return nc


_NC_CACHE = None


def _get_nc():
    global _NC_CACHE
    if _NC_CACHE is None:
        _NC_CACHE = build_nc()
    return _NC_CACHE


def _pad_t(x):
    xt = np.asarray(x).T.astype(ml_dtypes.bfloat16)
    out = np.zeros((128, xt.shape[1]), dtype=ml_dtypes.bfloat16)
    out[: xt.shape[0]] = xt
    return out


def make_in_maps(k_src, v_src, q_tgr, W_fc, b_fc):
    in_maps = []
    for core in range(N_CORES):
        n, h = divmod(core, 2)
        in_maps.append(
            {
                "qt": _pad_t(q_tgr[n, h * Q_SHARD : (h + 1) * Q_SHARD, :]),
                "kt": _pad_t(k_src[n]),
                "v": np.ascontiguousarray(np.asarray(v_src[n]).astype(ml_dtypes.bfloat16)),
                "wt": np.ascontiguousarray(
                    np.asarray(W_fc).T.astype(ml_dtypes.bfloat16)
                ),
                "b": np.ascontiguousarray(b_fc, dtype=np.float32),
            }
        )
    return in_maps


def assemble_out(results):
    out = np.empty((N_BATCH, P_KV, D_V), dtype=np.float32)
    for core in range(N_CORES):
        n, h = divmod(core, 2)
        out[n, h * Q_SHARD : (h + 1) * Q_SHARD, :] = results[core]["out"]
    return out


def kernel(k_src, v_src, q_tgr, W_fc, b_fc):
    from concourse.bass_utils import run_bass_kernel_spmd

    nc = _get_nc()
    in_maps = make_in_maps(k_src, v_src, q_tgr, W_fc, b_fc)
    res = run_bass_kernel_spmd(nc, in_maps, core_ids=list(range(N_CORES)))
    return assemble_out(res.results)
